# revision 1
# baseline (speedup 1.0000x reference)
"""CrossAttention Trainium2 kernel (8-core SPMD, batch x seq sharding).

Reference math (per batch b):
  q = x @ Wq ; k = ctx @ Wk ; v = ctx @ Wv        (heads H=16, dim_head D=64)
  scores = (q @ k^T) * D**-0.5 ; attn = softmax(scores, kv axis)
  out = (attn @ v) @ Wo + bo

Sharding: 8 cores = 4 batches x 2 halves of the query sequence (N=4096).
Each core computes one batch, 2048 queries, all 16 heads. K/V projections are
recomputed per n-half (2x replication, cheap). No collectives.

On-device layout is transposed: all host tensors are pre-transposed so that
contraction dims land on SBUF partitions. Dtypes: bf16 operands for
q/k/v/scores/AV matmuls (fp32 PSUM accumulation), fp32r for the output
projection, fp32 softmax normalization.
"""

from dataclasses import dataclass

import numpy as np
import ml_dtypes

import concourse.bass as bass
import concourse.mybir as mybir
import concourse.tile as tile
from concourse import bacc

F32 = mybir.dt.float32
F32R = mybir.dt.float32r
BF16 = mybir.dt.bfloat16
AF = mybir.ActivationFunctionType


@dataclass(frozen=True)
class Cfg:
    NB: int = 4      # n-blocks per core
    NW: int = 512    # n width per block (moving-operand width)
    FT: int = 8      # x feature tiles of 128 (QUERY_DIM/128)
    CT: int = 6      # ctx feature tiles of 128 (CONTEXT_DIM/128)
    H: int = 16      # heads
    D: int = 64      # dim per head
    MT: int = 8      # kv tiles of 128 (M/128)
    JT: int = 8      # output feature tiles of 128

    @property
    def HP(self):  # head pairs == q/k dcol tiles of 128
        return self.H // 2

    @property
    def M(self):
        return self.MT * 128

    @property
    def MW(self):  # m chunk width for kT matmuls
        return min(self.NW, self.M)

    @property
    def MC(self):
        return self.M // self.MW


FULL = Cfg()


def build_kernel(cfg: Cfg = FULL, dbg: bool = False):
    c = cfg
    nc = bacc.Bacc("TRN2", target_bir_lowering=False, debug=False)

    # DRAM I/O (per-core shapes)
    xT = nc.dram_tensor("xT", [c.NB, 128, c.FT, c.NW], BF16, kind="ExternalInput")
    ctxT = nc.dram_tensor("ctxT", [128, c.CT, c.M], BF16, kind="ExternalInput")
    wq = nc.dram_tensor("wq", [c.HP, 128, c.FT, 128], BF16, kind="ExternalInput")
    wk = nc.dram_tensor("wk", [c.HP, 128, c.CT, 128], BF16, kind="ExternalInput")
    wv = nc.dram_tensor("wv", [2, 128, c.CT, (c.H // 2) * c.D], BF16, kind="ExternalInput")
    wo = nc.dram_tensor("wo", [c.JT, 128, c.HP, 128], BF16, kind="ExternalInput")
    bo_t = nc.dram_tensor("bo_t", [128, c.JT], F32, kind="ExternalInput")
    outT = nc.dram_tensor("outT", [c.NB, 128, c.JT, c.NW], F32, kind="ExternalOutput")
    if dbg:
        dbg_q = nc.dram_tensor("dbg_q", [128, c.HP, c.NW], BF16, kind="ExternalOutput")
        dbg_k = nc.dram_tensor("dbg_k", [128, c.HP, c.M], BF16, kind="ExternalOutput")
        dbg_e = nc.dram_tensor("dbg_e", [2, 128, c.MT, c.NW], BF16, kind="ExternalOutput")
        dbg_av = nc.dram_tensor("dbg_av", [128, 2, c.NW], F32, kind="ExternalOutput")
        dbg_rbc = nc.dram_tensor("dbg_rbc", [64, 2, c.NW], F32, kind="ExternalOutput")
        dbg_r = nc.dram_tensor("dbg_r", [2, 2, c.NW], F32, kind="ExternalOutput")
        dbg_attn = nc.dram_tensor("dbg_attn", [64, c.H, c.NW], BF16, kind="ExternalOutput")

    VW = (c.H // 2) * c.D  # width of one v-projection half

    with tile.TileContext(nc) as tc:
        with (
            tc.tile_pool(name="persist", bufs=1) as persist,
            tc.tile_pool(name="wstream", bufs=3) as wstream,
            tc.tile_pool(name="nbuf", bufs=2) as nbuf,
            tc.tile_pool(name="hbuf", bufs=2) as hbuf,
            tc.tile_pool(name="nrm", bufs=2) as nrm,
            tc.tile_pool(name="obuf", bufs=1) as obuf,
            tc.tile_pool(name="dbounce", bufs=4, space="DRAM") as dbounce,
            tc.tile_pool(name="ps_acc", bufs=2, space="PSUM") as ps_acc,
            tc.tile_pool(name="ps_sc", bufs=2, space="PSUM") as ps_sc,
            tc.tile_pool(name="ps_av", bufs=2, space="PSUM") as ps_av,
        ):
            # ---- persistent tiles ----
            ctx_sb = persist.tile([128, c.CT, c.M], BF16)
            kT_all = persist.tile([128, c.HP, c.M], BF16)
            v_aug = persist.tile([128, c.MT, c.H, c.D + 1], BF16)
            bo_sb = persist.tile([128, c.JT], F32)

            nc.sync.dma_start(out=ctx_sb, in_=ctxT[:, :, :])
            nc.sync.dma_start(out=bo_sb, in_=bo_t[:, :])
            nc.vector.memset(v_aug[:, :, :, c.D : c.D + 1], 1.0)

            # ---- kT: kT_all[dpair, m] = (ctx @ Wk).T ----
            for dc in range(c.HP):
                wk_g = wstream.tile([128, c.CT, 128], BF16, tag="wk")
                nc.sync.dma_start(out=wk_g, in_=wk[dc])
                for mc in range(c.MC):
                    ps = ps_acc.tile([128, c.MW], F32, tag="acc")
                    msl = bass.ts(mc, c.MW)
                    for ct in range(c.CT):
                        nc.tensor.matmul(
                            ps[:, :], wk_g[:, ct, :], ctx_sb[:, ct, msl],
                            start=(ct == 0), stop=(ct == c.CT - 1),
                        )
                    nc.vector.tensor_copy(out=kT_all[:, dc, msl], in_=ps[:, :])

            # ---- v: v_aug[m_tile, h, 0:D] = ctx @ Wv (strided into aug) ----
            for dh in range(2):
                wv_g = wstream.tile([128, c.CT, VW], BF16, tag="wv")
                nc.sync.dma_start(out=wv_g, in_=wv[dh])
                for mt in range(c.MT):
                    ps = ps_acc.tile([128, VW], F32, tag="acc")
                    for ct in range(c.CT):
                        nc.tensor.matmul(
                            ps[:, :], ctx_sb[:, ct, bass.ts(mt, 128)], wv_g[:, ct, :],
                            start=(ct == 0), stop=(ct == c.CT - 1),
                        )
                    nc.vector.tensor_copy(
                        out=v_aug[:, mt, bass.ts(dh, c.H // 2), 0 : c.D],
                        in_=ps[:, :].rearrange("p (h d) -> p h d", d=c.D),
                    )

            # ---- per n-block ----
            for nb in range(c.NB):
                x_sb = nbuf.tile([128, c.FT, c.NW], BF16, tag="x", bufs=1)
                nc.sync.dma_start(out=x_sb, in_=xT[nb])

                # qT_all[dpair, n] = (x @ Wq).T, Wq pre-scaled by D**-0.5
                qT_all = nbuf.tile([128, c.HP, c.NW], BF16, tag="qT", bufs=1)
                for dc in range(c.HP):
                    wq_g = wstream.tile([128, c.FT, 128], BF16, tag="wq")
                    nc.sync.dma_start(out=wq_g, in_=wq[dc])
                    ps = ps_acc.tile([128, c.NW], F32, tag="acc")
                    for ft in range(c.FT):
                        nc.tensor.matmul(
                            ps[:, :], wq_g[:, ft, :], x_sb[:, ft, :],
                            start=(ft == 0), stop=(ft == c.FT - 1),
                        )
                    nc.vector.tensor_copy(out=qT_all[:, dc, :], in_=ps[:, :])

                attn_n = nrm.tile([128, c.HP, c.NW], BF16, tag="attn", bufs=1)

                for hp in range(c.HP):
                    av = ps_av.tile([128, 2, c.NW], F32, tag="av")
                    exp_ev = hbuf.tile([128, c.MT, c.NW], BF16, tag="exp")
                    exp_od = hbuf.tile([128, c.MT, c.NW], BF16, tag="expo")
                    exp_p = [exp_ev, exp_od]
                    # scores^T [m, n] for both heads of the pair, interleaved so
                    # the K=64 matmuls co-run on disjoint PE row groups
                    for mt in range(c.MT):
                        for par in range(2):
                            prow = slice(par * 64, par * 64 + 64)
                            ps = ps_sc.tile([128, c.NW], F32, tag="sc")
                            nc.tensor.matmul(
                                ps[:, :],
                                kT_all[prow, hp, bass.ts(mt, 128)],
                                qT_all[prow, hp, :],
                                start=True, stop=True,
                            )
                            nc.scalar.activation(
                                out=exp_p[par][:, mt, :], in_=ps[:, :], func=AF.Exp,
                            )
                    if dbg and nb == 0 and hp == 0:
                        nc.sync.dma_start(out=dbg_e[0], in_=exp_p[0][:, :, :])
                        nc.sync.dma_start(out=dbg_e[1], in_=exp_p[1][:, :, :])
                    # AV + row sums (ones column): [D+1, NW] into psum pair
                    for par in range(2):
                        h = 2 * hp + par
                        for mt in range(c.MT):
                            nc.tensor.matmul(
                                av[0 : c.D + 1, par, :],
                                v_aug[:, mt, h, :],
                                exp_p[par][:, mt, :],
                                start=(mt == 0), stop=(mt == c.MT - 1),
                            )
                    # normalize both heads of the pair
                    r_sb = nrm.tile([128, 2, c.NW], F32, tag="r", bufs=3)
                    nc.scalar.activation(
                        out=r_sb[c.D : c.D + 1, :, :],
                        in_=av[c.D : c.D + 1, :, :].rearrange("p a n -> p (a n)"),
                        func=AF.Ln,
                    )
                    nc.scalar.activation(
                        out=r_sb[c.D : c.D + 1, :, :],
                        in_=r_sb[c.D : c.D + 1, :, :].rearrange("p a n -> p (a n)"),
                        func=AF.Exp, scale=-1.0,
                    )
                    r_dram = dbounce.tile([1, 2, c.NW], F32, tag="rd")
                    nc.sync.dma_start(out=r_dram, in_=r_sb[c.D : c.D + 1, :, :])
                    r_bc = nrm.tile([64, 2, c.NW], F32, tag="rbc", bufs=3)
                    nc.sync.dma_start(
                        out=r_bc, in_=r_dram[:, :, :].to_broadcast([64, 2, c.NW])
                    )
                    nc.vector.tensor_mul(
                        out=attn_n[0:64, hp, :],
                        in0=av[0:64, 0, :],
                        in1=r_bc[:, 0, :],
                    )
                    sh_tmp = nrm.tile([64, c.NW], BF16, tag="sh", bufs=2)
                    nc.vector.tensor_mul(
                        out=sh_tmp[:, :],
                        in0=av[0:64, 1, :],
                        in1=r_bc[:, 1, :],
                    )
                    nc.sync.dma_start(out=attn_n[64:128, hp, :], in_=sh_tmp[:, :])
                    if dbg and nb == 0 and hp == 0:
                        av_dump = nrm.tile([128, 2, c.NW], F32, tag="avd", bufs=1)
                        nc.vector.tensor_copy(out=av_dump[0:65, :, :], in_=av[0:65, :, :])
                        nc.sync.dma_start(out=dbg_av[0:65, :, :], in_=av_dump[0:65, :, :])
                        nc.sync.dma_start(out=dbg_rbc[:, :, :], in_=r_bc)
                        nc.sync.dma_start(out=dbg_r[0:1], in_=s_row[c.D : c.D + 1, :, :])
                        nc.sync.dma_start(out=dbg_r[1:2], in_=r_sb[c.D : c.D + 1, :, :])

                if dbg and nb == 0:
                    nc.sync.dma_start(out=dbg_q[:, :, :], in_=qT_all[:, :, :])
                    nc.sync.dma_start(out=dbg_k[:, :, :], in_=kT_all[:, :, :])
                # output projection (fp32r) + bias
                for j in range(c.JT):
                    wo_g = wstream.tile([128, c.HP, 128], BF16, tag="wo")
                    nc.sync.dma_start(out=wo_g, in_=wo[j])
                    ps = ps_acc.tile([128, c.NW], F32, tag="acc")
                    for hp2 in range(c.HP):
                        nc.tensor.matmul(
                            ps[:, :], wo_g[:, hp2, :],
                            attn_n[:, hp2, :],
                            start=(hp2 == 0), stop=(hp2 == c.HP - 1),
                        )
                    out_sb = obuf.tile([128, c.NW], F32, tag="out", bufs=2)
                    nc.vector.tensor_scalar_add(
                        out=out_sb[:, :], in0=ps[:, :], scalar1=bo_sb[:, j : j + 1]
                    )
                    nc.sync.dma_start(out=outT[nb][:, j, :], in_=out_sb)

    nc.compile()
    return nc


# ---------------- host side ----------------

def _prep_inputs(x, context, Wq, Wk, Wv, Wo, bo, cfg: Cfg = FULL, n_cores: int = 8):
    """Build the 8 per-core input maps (host-side transposes)."""
    c = cfg
    bf = ml_dtypes.bfloat16
    scale = np.float32(c.D) ** np.float32(-0.5)
    QD, CD, INNER, OD = c.FT * 128, c.CT * 128, c.H * c.D, c.JT * 128
    NCORE = c.NB * c.NW

    wq_t = np.ascontiguousarray(
        (Wq.astype(np.float32) * scale).reshape(c.FT, 128, c.HP, 128).transpose(2, 1, 0, 3)
    ).astype(bf)
    wk_t = np.ascontiguousarray(
        Wk.reshape(c.CT, 128, c.HP, 128).transpose(2, 1, 0, 3)
    ).astype(bf)
    wv_t = np.ascontiguousarray(
        Wv.reshape(c.CT, 128, 2, (c.H // 2) * c.D).transpose(2, 1, 0, 3)
    ).astype(bf)
    # rows hd of Wo grouped as [hp][par*64+d]: row index = (2*hp+par)*64+d
    wo_t = np.ascontiguousarray(
        Wo.reshape(c.HP, 2 * c.D, c.JT, 128).transpose(2, 1, 0, 3)
    ).astype(bf)
    bo_tt = np.ascontiguousarray(bo.reshape(c.JT, 128).T).astype(np.float32)

    B = x.shape[0]
    n_halves = n_cores // B
    in_maps = []
    for core in range(n_cores):
        b = core // n_halves
        n0 = (core % n_halves) * NCORE
        xs = x[b, n0 : n0 + NCORE, :]  # [NCORE, QD]
        xT_c = np.ascontiguousarray(
            xs.reshape(c.NB, c.NW, c.FT, 128).transpose(0, 3, 2, 1)
        ).astype(bf)
        ctxT_c = np.ascontiguousarray(
            context[b].T.reshape(c.CT, 128, c.M).transpose(1, 0, 2)
        ).astype(bf)
        in_maps.append({
            "xT": xT_c, "ctxT": ctxT_c, "wq": wq_t, "wk": wk_t,
            "wv": wv_t, "wo": wo_t, "bo_t": bo_tt,
        })
    return in_maps


def _gather_output(results, B, N, cfg: Cfg = FULL, n_cores: int = 8):
    c = cfg
    OD = c.JT * 128
    NCORE = c.NB * c.NW
    n_halves = n_cores // B
    out = np.empty((B, N, OD), dtype=np.float32)
    for core in range(n_cores):
        b = core // n_halves
        n0 = (core % n_halves) * NCORE
        oT = results[core]["outT"]  # [NB, 128, JT, NW]
        out[b, n0 : n0 + NCORE, :] = (
            oT.transpose(0, 3, 2, 1).reshape(NCORE, OD)
        )
    return out


_NC_CACHE = {}


def kernel(x, context, Wq, Wk, Wv, Wo, bo):
    from concourse.bass_utils import run_bass_kernel_spmd

    cfg = FULL
    if "nc" not in _NC_CACHE:
        _NC_CACHE["nc"] = build_kernel(cfg)
    nc = _NC_CACHE["nc"]

    x = np.asarray(x, dtype=np.float32)
    context = np.asarray(context, dtype=np.float32)
    in_maps = _prep_inputs(
        x, context,
        np.asarray(Wq, np.float32), np.asarray(Wk, np.float32),
        np.asarray(Wv, np.float32), np.asarray(Wo, np.float32),
        np.asarray(bo, np.float32), cfg,
    )
    res = run_bass_kernel_spmd(nc, in_maps, core_ids=list(range(8)))
    return _gather_output(res.results, x.shape[0], x.shape[1], cfg)



# revision 12
# speedup vs baseline: 1.4450x; 1.4450x over previous
"""CrossAttention Trainium2 kernel (8-core SPMD, batch x seq sharding).

Reference math (per batch b):
  q = x @ Wq ; k = ctx @ Wk ; v = ctx @ Wv        (heads H=16, dim_head D=64)
  scores = (q @ k^T) * D**-0.5 ; attn = softmax(scores, kv axis)
  out = (attn @ v) @ Wo + bo

Sharding: 8 cores = 4 batches x 2 halves of the query sequence (N=4096).
Each core computes one batch, 2048 queries, all 16 heads. K/V projections are
recomputed per n-half (2x replication, cheap). No collectives.

On-device layout is transposed: all host tensors are pre-transposed so that
contraction dims land on SBUF partitions. Softmax row sums come from an
augmented ones-column in the V stationary; the even head of each pair lands on
PSUM partitions 0-64 (sum at 64) and the odd head on partitions 63-127 (sum at
63, zero-padded stationary below), so the normalized pair assembles into the
128-partition attn tile without any partition-moving DMA. Normalizers are
reciprocal'd on DVE and broadcast across partitions on the (otherwise idle)
GPSIMD/Pool engine.
"""

from dataclasses import dataclass

import numpy as np
import ml_dtypes

import concourse.bass as bass
import concourse.mybir as mybir
import concourse.tile as tile
from concourse import bacc

F32 = mybir.dt.float32
BF16 = mybir.dt.bfloat16
AF = mybir.ActivationFunctionType


@dataclass(frozen=True)
class Cfg:
    NB: int = 4      # n-blocks per core
    NW: int = 512    # n width per block (moving-operand width)
    FT: int = 8      # x feature tiles of 128 (QUERY_DIM/128)
    CT: int = 6      # ctx feature tiles of 128 (CONTEXT_DIM/128)
    H: int = 16      # heads
    D: int = 64      # dim per head
    MT: int = 8      # kv tiles of 128 (M/128)
    JT: int = 8      # output feature tiles of 128

    @property
    def HP(self):  # head pairs == q/k dcol tiles of 128
        return self.H // 2

    @property
    def M(self):
        return self.MT * 128


FULL = Cfg()


def build_kernel(cfg: Cfg = FULL, dbg: bool = False):
    c = cfg
    nc = bacc.Bacc("TRN2", target_bir_lowering=False, debug=False)
    VW = (c.H // 2) * c.D  # 512: one parity's worth of v columns

    # DRAM I/O (per-core shapes, partition-major so each loads as one DMA)
    xT = nc.dram_tensor("xT", [c.NB, 128, c.FT, c.NW], BF16, kind="ExternalInput")
    ctxT = nc.dram_tensor("ctxT", [128, c.CT, c.M], BF16, kind="ExternalInput")
    wq = nc.dram_tensor("wq", [128, c.HP, c.FT, 128], BF16, kind="ExternalInput")
    wk = nc.dram_tensor("wk", [128, c.HP, c.CT, 128], BF16, kind="ExternalInput")
    wv = nc.dram_tensor("wv", [128, 2, c.CT, VW], BF16, kind="ExternalInput")
    wo = nc.dram_tensor("wo", [128, c.JT, c.HP, 128], BF16, kind="ExternalInput")
    bo_t = nc.dram_tensor("bo_t", [128, c.JT], F32, kind="ExternalInput")
    outT = nc.dram_tensor("outT", [c.NB, 128, c.JT, c.NW], F32, kind="ExternalOutput")

    with tile.TileContext(nc) as tc:
        with (
            tc.tile_pool(name="persist", bufs=1) as persist,
            tc.tile_pool(name="nbuf", bufs=2) as nbuf,
            tc.tile_pool(name="hbuf", bufs=2) as hbuf,
            tc.tile_pool(name="nrm", bufs=2) as nrm,
            tc.tile_pool(name="obuf", bufs=2) as obuf,
            tc.tile_pool(name="ps_sc", bufs=2, space="PSUM") as ps_sc,
            tc.tile_pool(name="ps_av", bufs=2, space="PSUM") as ps_av,
        ):
            # ---- persistent tiles ----
            ctx_sb = persist.tile([128, c.CT, c.M], BF16)
            kT_all = persist.tile([128, c.HP, c.M], BF16)
            # v stationaries: parity 0 (even heads) cols [v(64), ones];
            # parity 1 (odd heads) cols [zeros(63), ones, v(64)] so its AV
            # lands on PSUM partitions 64..127 with the row-sum at 63.
            vaug0 = persist.tile([128, c.MT, c.HP, c.D + 1], BF16)
            vaug1 = persist.tile([128, c.MT, c.HP, 128], BF16)
            bo_sb = persist.tile([128, c.JT], F32)
            wq_sb = persist.tile([128, c.HP, c.FT, 128], BF16)
            wk_sb = persist.tile([128, c.HP, c.CT, 128], BF16)
            wv_sb = persist.tile([128, 2, c.CT, VW], BF16)
            wo_sb = persist.tile([128, c.JT, c.HP, 128], BF16)

            # startup DMAs ordered so the first kT matmuls can begin early:
            # wk/ctx chunks interleaved, everything else after
            for i in range(c.HP):
                nc.sync.dma_start(out=wk_sb[:, i, :, :], in_=wk[:, i, :, :])
                if i < c.CT:
                    nc.sync.dma_start(out=ctx_sb[:, i, :], in_=ctxT[:, i, :])
            xs = {0: nbuf.tile([128, c.FT, c.NW], BF16, tag="x")}
            nc.sync.dma_start(out=xs[0], in_=xT[0])
            nc.sync.dma_start(out=wq_sb, in_=wq[:, :, :, :])
            nc.sync.dma_start(out=wv_sb, in_=wv[:, :, :, :])
            nc.sync.dma_start(out=wo_sb, in_=wo[:, :, :, :])
            nc.sync.dma_start(out=bo_sb, in_=bo_t[:, :])
            nc.vector.memset(vaug0[:, :, :, c.D : c.D + 1], 1.0)
            nc.vector.memset(vaug1[:, :, :, 0:1], 1.0)
            nc.gpsimd.memset(vaug1[:, :, :, 1 : c.D], 0.0)

            # ---- kT: kT_all[dpair, m] = (ctx @ Wk).T ----
            for dc in range(c.HP):
                ps = ps_sc.tile([128, 2, c.NW], F32, tag="sc")
                for mc in range(2):
                    for ct in range(c.CT):
                        nc.tensor.matmul(
                            ps[:, mc, :],
                            wk_sb[:, dc, ct, :],
                            ctx_sb[:, ct, bass.ts(mc, c.NW)],
                            start=(ct == 0), stop=(ct == c.CT - 1),
                        )
                nc.vector.tensor_copy(
                    out=kT_all[:, dc, :],
                    in_=ps[:, :, :].rearrange("p a n -> p (a n)"),
                )

            # ---- v: vaug{par}[m_tile, hp, :] = ctx @ Wv (parity-split) ----
            for dh in range(2):
                voff = 0 if dh == 0 else c.D
                vdst = vaug0 if dh == 0 else vaug1
                for mtp in range(c.MT // 2):
                    ps = ps_sc.tile([128, 2, c.NW], F32, tag="sc")
                    for k in range(2):
                        mt = 2 * mtp + k
                        for ct in range(c.CT):
                            nc.tensor.matmul(
                                ps[:, k, :],
                                ctx_sb[:, ct, bass.ts(mt, 128)],
                                wv_sb[:, dh, ct, :],
                                start=(ct == 0), stop=(ct == c.CT - 1),
                            )
                    nc.vector.tensor_copy(
                        out=vdst[:, 2 * mtp : 2 * mtp + 2, :, voff : voff + c.D],
                        in_=ps[:, :, :].rearrange("p a (h d) -> p a h d", d=c.D),
                    )

            # ---- per n-block ----
            for nb in range(c.NB):
                x_sb = nbuf.tile([128, c.FT, c.NW], BF16, tag="x")
                nc.sync.dma_start(out=x_sb, in_=xT[nb])

                # qT_all[dpair, n] = (x @ Wq).T, Wq pre-scaled by D**-0.5
                qT_all = nbuf.tile([128, c.HP, c.NW], BF16, tag="qT", bufs=1)
                for dcp in range(c.HP // 2):
                    ps = ps_sc.tile([128, 2, c.NW], F32, tag="sc")
                    for k in range(2):
                        dc = 2 * dcp + k
                        for ft in range(c.FT):
                            nc.tensor.matmul(
                                ps[:, k, :], wq_sb[:, dc, ft, :], x_sb[:, ft, :],
                                start=(ft == 0), stop=(ft == c.FT - 1),
                            )
                    nc.vector.tensor_copy(
                        out=qT_all[:, 2 * dcp : 2 * dcp + 2, :],
                        in_=ps[:, :, :].rearrange("p a n -> p (a n)"),
                    )

                attn_n = nrm.tile([128, c.HP, c.NW], BF16, tag="attn")

                for hp in range(c.HP):
                    exp_ev = hbuf.tile([128, c.MT, c.NW], BF16, tag="exp")
                    exp_od = hbuf.tile([128, c.MT, c.NW], BF16, tag="expo")
                    exp_p = [exp_ev, exp_od]
                    # scores^T [m, n] per parity; exp fused into the
                    # PSUM->SBUF move on ACT, two banks per instruction
                    for par in range(2):
                        prow = slice(par * 64, par * 64 + 64)
                        for mtp in range(c.MT // 2):
                            ps = ps_sc.tile([128, 2, c.NW], F32, tag="sc")
                            for k in range(2):
                                mt = 2 * mtp + k
                                nc.tensor.matmul(
                                    ps[:, k, :],
                                    kT_all[prow, hp, bass.ts(mt, 128)],
                                    qT_all[prow, hp, :],
                                    start=True, stop=True,
                                )
                            nc.scalar.activation(
                                out=exp_p[par][:, 2 * mtp : 2 * mtp + 2, :].rearrange(
                                    "p a n -> p (a n)"
                                ),
                                in_=ps[:, :, :].rearrange("p a n -> p (a n)"),
                                func=AF.Exp,
                            )
                    # AV + row sums (ones column): parity 0 -> psum rows 0..64,
                    # parity 1 -> rows 63..127 (63 = sum)
                    av = ps_av.tile([128, 2, c.NW], F32, tag="av")
                    for mt in range(c.MT):
                        nc.tensor.matmul(
                            av[0 : c.D + 1, 0, :],
                            vaug0[:, mt, hp, :],
                            exp_ev[:, mt, :],
                            start=(mt == 0), stop=(mt == c.MT - 1),
                        )
                    for mt in range(c.MT):
                        nc.tensor.matmul(
                            av[:, 1, :],
                            vaug1[:, mt, hp, :],
                            exp_od[:, mt, :],
                            start=(mt == 0), stop=(mt == c.MT - 1),
                        )
                    # normalizers: even head's sum sits at psum partition 64 —
                    # DMA it to a partition-0 staging row (HW partition_broadcast
                    # only reads/writes from partition 0); odd head's sum lands
                    # on partition 0 natively (ones in stationary col 0).
                    r_sb = nrm.tile([128, c.NW], F32, tag="r", bufs=2)
                    nc.vector.reciprocal(out=r_sb[64:65, :], in_=av[64:65, 0, :])
                    nc.vector.reciprocal(out=r_sb[0:1, :], in_=av[0:1, 1, :])
                    rst = nrm.tile([1, c.NW], F32, tag="rst", bufs=2)
                    nc.gpsimd.dma_start(out=rst[0:1, :], in_=r_sb[64:65, :])
                    r_bc = nrm.tile([128, 2, c.NW], F32, tag="rbc", bufs=2)
                    nc.gpsimd.partition_broadcast(
                        r_bc[:, 0, :], rst[0:1, :], channels=128
                    )
                    nc.gpsimd.partition_broadcast(
                        r_bc[:, 1, :], r_sb[0:1, :], channels=128
                    )
                    nc.vector.tensor_mul(
                        out=attn_n[0:64, hp, :],
                        in0=av[0:64, 0, :],
                        in1=r_bc[0:64, 0, :],
                    )
                    nc.vector.tensor_mul(
                        out=attn_n[64:128, hp, :],
                        in0=av[64:128, 1, :],
                        in1=r_bc[64:128, 1, :],
                    )

                # output projection + bias
                for jp in range(c.JT // 2):
                    ps = ps_av.tile([128, 2, c.NW], F32, tag="av")
                    for k in range(2):
                        j = 2 * jp + k
                        for hp2 in range(c.HP):
                            nc.tensor.matmul(
                                ps[:, k, :], wo_sb[:, j, hp2, :],
                                attn_n[:, hp2, :],
                                start=(hp2 == 0), stop=(hp2 == c.HP - 1),
                            )
                    out_sb = obuf.tile([128, 2, c.NW], F32, tag="out")
                    for k in range(2):
                        j = 2 * jp + k
                        nc.vector.tensor_scalar_add(
                            out=out_sb[:, k, :], in0=ps[:, k, :],
                            scalar1=bo_sb[:, j : j + 1],
                        )
                    nc.sync.dma_start(
                        out=outT[nb][:, 2 * jp : 2 * jp + 2, :], in_=out_sb
                    )

    nc.compile()
    return nc


# ---------------- host side ----------------

def _prep_inputs(x, context, Wq, Wk, Wv, Wo, bo, cfg: Cfg = FULL, n_cores: int = 8):
    """Build the 8 per-core input maps (host-side transposes)."""
    c = cfg
    bf = ml_dtypes.bfloat16
    scale = np.float32(c.D) ** np.float32(-0.5)
    NCORE = c.NB * c.NW

    # [128, HP, FT, 128]: partition = head-pair row (par*64+d of q contraction)
    wq_t = np.ascontiguousarray(
        (Wq.astype(np.float32) * scale)
        .reshape(c.FT, 128, c.HP, 128).transpose(1, 2, 0, 3)
    ).astype(bf)
    wk_t = np.ascontiguousarray(
        Wk.reshape(c.CT, 128, c.HP, 128).transpose(1, 2, 0, 3)
    ).astype(bf)
    # [128, 2, CT, VW]: parity-split v columns (par = head % 2)
    wv_t = np.ascontiguousarray(
        Wv.reshape(c.CT, 128, c.HP, 2, c.D).transpose(1, 3, 0, 2, 4)
        .reshape(128, 2, c.CT, c.HP * c.D)
    ).astype(bf)
    # [128, JT, HP, 128]: partition = par*64+d row of attn
    wo_t = np.ascontiguousarray(
        Wo.reshape(c.HP, 2, c.D, c.JT, 128).transpose(1, 2, 3, 0, 4)
        .reshape(128, c.JT, c.HP, 128)
    ).astype(bf)
    bo_tt = np.ascontiguousarray(bo.reshape(c.JT, 128).T).astype(np.float32)

    B = x.shape[0]
    n_halves = n_cores // B
    in_maps = []
    for core in range(n_cores):
        b = core // n_halves
        n0 = (core % n_halves) * NCORE
        xs = x[b, n0 : n0 + NCORE, :]  # [NCORE, QD]
        xT_c = np.ascontiguousarray(
            xs.reshape(c.NB, c.NW, c.FT, 128).transpose(0, 3, 2, 1)
        ).astype(bf)
        ctxT_c = np.ascontiguousarray(
            context[b].T.reshape(c.CT, 128, c.M).transpose(1, 0, 2)
        ).astype(bf)
        in_maps.append({
            "xT": xT_c, "ctxT": ctxT_c, "wq": wq_t, "wk": wk_t,
            "wv": wv_t, "wo": wo_t, "bo_t": bo_tt,
        })
    return in_maps


def _gather_output(results, B, N, cfg: Cfg = FULL, n_cores: int = 8):
    c = cfg
    OD = c.JT * 128
    NCORE = c.NB * c.NW
    n_halves = n_cores // B
    out = np.empty((B, N, OD), dtype=np.float32)
    for core in range(n_cores):
        b = core // n_halves
        n0 = (core % n_halves) * NCORE
        oT = results[core]["outT"]  # [NB, 128, JT, NW]
        out[b, n0 : n0 + NCORE, :] = (
            oT.transpose(0, 3, 2, 1).reshape(NCORE, OD)
        )
    return out


_NC_CACHE = {}


def kernel(x, context, Wq, Wk, Wv, Wo, bo):
    from concourse.bass_utils import run_bass_kernel_spmd

    cfg = FULL
    if "nc" not in _NC_CACHE:
        _NC_CACHE["nc"] = build_kernel(cfg)
    nc = _NC_CACHE["nc"]

    x = np.asarray(x, dtype=np.float32)
    context = np.asarray(context, dtype=np.float32)
    in_maps = _prep_inputs(
        x, context,
        np.asarray(Wq, np.float32), np.asarray(Wk, np.float32),
        np.asarray(Wv, np.float32), np.asarray(Wo, np.float32),
        np.asarray(bo, np.float32), cfg,
    )
    res = run_bass_kernel_spmd(nc, in_maps, core_ids=list(range(8)))
    return _gather_output(res.results, x.shape[0], x.shape[1], cfg)
